# revision 10
# baseline (speedup 1.0000x reference)
"""Trainium2 Bass kernel for CustomStellarEncoder (2x dense+BN+relu, 2x SAGEConv+BN).

Strategy (8 NeuronCores, SPMD):
  - Nodes partitioned contiguously across cores (6250/core).
  - Activations kept feature-major ([128 feat partitions, nodes free]) in bf16; all
    dense matmuls bf16 (PSUM f32). Linear biases dropped (BatchNorm absorbs them).
  - SAGE mean-aggregation: edges bucketed host-side by (dst block of 128, src
    table a/b), padded to 128-edge chunks, batched into big dma_gather calls
    (SWDGE). Per chunk the gathered source rows G [128e x 128f] (bf16) are
    contracted on TensorE with a one-hot P [128e x 128slot] built on VectorE:
    matmul(lhsT=G, rhs=P) accumulates meanT [feat x slot] directly in PSUM
    (feature-major, no drain transpose); drain scales by inv_cnt (bf16 row tile).
  - Halo exchange: node-major bf16 shards AllGathered in two halves (table a =
    blocks 0..24, table b = blocks 25..48) so gathers can start after the first.
  - Layer-2 aggregates PRE-BN values: BN3 is affine, so A*BN3(x) =
    sc3*(A*x) + sh3; sc3 is folded into Wl2 on device and the sh3 constant is
    absorbed exactly by BN4 (all nodes have in-edges). This unhooks layer-2's
    gather from the BN3 AllReduce.
  - Global BN statistics via AllReduce of [128,2] partial (sum, sumsq).
"""

import os
import sys
import numpy as np

sys.path.insert(0, "/opt/trn_rl_repo")

import ml_dtypes


def _install_ntff_hook_shim():
    """The agent image's `antenv` lacks `axon_hooks`; bass_utils imports it
    unconditionally when trace=True under axon. Provide it, registering the
    same ctypes NTFF hook trn_agent_boot would have installed."""
    import types
    if "antenv.axon_hooks" in sys.modules:
        return
    mod = types.ModuleType("antenv.axon_hooks")
    state = {"hook": None}
    mod.set_axon_ntff_profile_hook = lambda h: state.__setitem__("hook", h)
    mod.get_axon_ntff_profile_hook = lambda: state["hook"]
    try:
        import antenv
        sys.modules["antenv.axon_hooks"] = mod
        antenv.axon_hooks = mod
        from trn_agent_boot.trn_boot import _ntff_profile_via_ctypes
        mod.set_axon_ntff_profile_hook(
            _ntff_profile_via_ctypes("/opt/axon/libaxon_pjrt.so"))
    except Exception as e:  # tracing degrades; execution still works
        print(f"ntff hook shim unavailable: {e}", file=sys.stderr)


_install_ntff_hook_shim()

N = 50000
E = 1600000
IN_DIM = 256
HID = 128
NCORES = 8
EPS = 1e-5

NPC = N // NCORES            # 6250
NB = -(-NPC // 128)          # 49 blocks
NPAD = NB * 128              # 6272
ABLK = 25                    # table-a blocks per core
ASZ = ABLK * 128             # 3200 nodes/core in table a
BSZ = NPC - ASZ              # 3050 nodes/core in table b

GRP = 4      # dst blocks whose PSUM accumulators are live together
GMAX = 8     # max 128-edge chunks per dma_gather call (HW SWDGE ring limit)
NQ = 4       # SWDGE queues
SCRATCH = 16384
DENSE_CHUNK = 512

BF16 = ml_dtypes.bfloat16


# ---------------------------------------------------------------- host prep

def _prep(edge_index):
    src = edge_index[0].astype(np.int64)
    dst = edge_index[1].astype(np.int64)

    cnt = np.bincount(dst, minlength=N)
    assert (cnt > 0).all(), "bn3 shift absorption requires all nodes to have in-edges"
    invc = (1.0 / np.maximum(cnt, 1)).astype(np.float32)

    sc_ = src // NPC
    sl = src % NPC
    tbl = (sl >= ASZ).astype(np.int64)
    lidx = np.where(tbl == 0, sc_ * ASZ + sl, sc_ * BSZ + (sl - ASZ))

    core = dst // NPC
    block = (dst % NPC) // 128
    dloc = (dst % NPC) % 128

    bucket = (core * NB + block) * 2 + tbl
    order = np.argsort(bucket, kind="stable")
    bc = np.bincount(bucket, minlength=NCORES * NB * 2).reshape(NCORES, NB, 2)
    ch = -(-bc.max(axis=0) // 128)  # [NB, 2] chunks per (block, table)

    stream = []   # chunk idx -> block
    calls = []    # (table, q0_chunk, nch)
    bstart_chunk = {}
    q = 0
    for g0 in range(0, NB, GRP):
        gb = list(range(g0, min(g0 + GRP, NB)))
        for t in (0, 1):
            run = []
            for b in gb:
                bstart_chunk[(b, t)] = q + len(run)
                run += [b] * int(ch[b, t])
            off = 0
            while off < len(run):
                c = min(GMAX, len(run) - off)
                calls.append((t, q + off, c))
                off += c
            stream += run
            q += len(run)
    tch = q
    first_chunk = {}
    last_chunk = {}
    for ci, b in enumerate(stream):
        first_chunk.setdefault(b, ci)
        last_chunk[b] = ci

    bstarts = np.zeros(NCORES * NB * 2 + 1, np.int64)
    np.cumsum(bc.reshape(-1), out=bstarts[1:])

    slots = tch * 128
    idx_all = np.zeros((NCORES, slots), np.int16)
    dl_all = np.full((NCORES, slots), 200.0, np.float32)
    for c in range(NCORES):
        for b in range(NB):
            for t in (0, 1):
                bid = (c * NB + b) * 2 + t
                e0, e1 = int(bstarts[bid]), int(bstarts[bid + 1])
                m = e1 - e0
                if m == 0:
                    continue
                so = bstart_chunk[(b, t)] * 128
                ids = order[e0:e1]
                idx_all[c, so:so + m] = lidx[ids].astype(np.int16)
                dl_all[c, so:so + m] = dloc[ids]

    per_core = []
    for c in range(NCORES):
        idx_w = np.tile(np.ascontiguousarray(idx_all[c].reshape(-1, 16).T), (8, 1))
        ivp = np.zeros(NPAD, np.float32)
        ivp[:NPC] = invc[c * NPC:(c + 1) * NPC]
        invc_bc = np.broadcast_to(ivp[None, :], (128, NPAD)).astype(BF16)
        per_core.append({
            "idx": np.ascontiguousarray(idx_w),
            "dloc": np.ascontiguousarray(dl_all[c].reshape(tch, 128).T).astype(BF16),
            "invc": np.ascontiguousarray(invc_bc),
        })

    meta = {
        "tch": tch,
        "calls": tuple(calls),
        "stream": tuple(stream),
        "first_chunk": first_chunk,
        "last_chunk": last_chunk,
    }
    return meta, per_core


# ---------------------------------------------------------------- device build

def _build(meta):
    import concourse.bacc as bacc
    import concourse.tile as tile
    from concourse import mybir

    f32 = mybir.dt.float32
    bf16 = mybir.dt.bfloat16
    i16 = mybir.dt.int16
    ADD = mybir.AluOpType.add
    MUL = mybir.AluOpType.mult
    SUB = mybir.AluOpType.subtract
    ISEQ = mybir.AluOpType.is_equal
    BYP = mybir.AluOpType.bypass
    AX = mybir.AxisListType.X
    AF = mybir.ActivationFunctionType

    tch = meta["tch"]
    calls = meta["calls"]
    stream = meta["stream"]
    first_chunk = meta["first_chunk"]
    last_chunk = meta["last_chunk"]
    cols = tch * 8
    inv_n = 1.0 / N
    rg = [list(range(NCORES))]

    cks = [(i, min(DENSE_CHUNK, NPC - i)) for i in range(0, NPC, DENSE_CHUNK)]
    nck = len(cks)
    # x-transpose chunks cover the padded 6272 rows
    xks = [(i, min(1568, NPAD - i)) for i in range(0, NPAD, 1568)]

    nc = bacc.Bacc("TRN2", target_bir_lowering=False, debug=False,
                   num_devices=NCORES, num_swdge_queues=NQ,
                   dynamic_dma_scratch_size=SCRATCH)

    # -------- I/O
    xs_t = nc.dram_tensor("xs", [2, NPAD, 128], bf16, kind="ExternalInput")
    idx_t = nc.dram_tensor("idx", [128, cols], i16, kind="ExternalInput")
    dloc_t = nc.dram_tensor("dloc", [128, tch], bf16, kind="ExternalInput")
    invc_t = nc.dram_tensor("invc", [128, NPAD], bf16, kind="ExternalInput")
    w_in_t = nc.dram_tensor("w_in", [2, 128, HID], bf16, kind="ExternalInput")
    w_hid_t = nc.dram_tensor("w_hid", [HID, HID], bf16, kind="ExternalInput")
    wl1_t = nc.dram_tensor("wl1", [HID, HID], bf16, kind="ExternalInput")
    wr1_t = nc.dram_tensor("wr1", [HID, HID], bf16, kind="ExternalInput")
    wl2_t = nc.dram_tensor("wl2", [HID, HID], bf16, kind="ExternalInput")
    wr2_t = nc.dram_tensor("wr2", [HID, HID], bf16, kind="ExternalInput")
    gb_t = nc.dram_tensor("gb", [128, 8], f32, kind="ExternalInput")
    iota_t = nc.dram_tensor("iota", [128, GMAX * 128], bf16, kind="ExternalInput")
    ident_t = nc.dram_tensor("ident", [128, 128], bf16, kind="ExternalInput")
    feat_o = nc.dram_tensor("feat_o", [NPC, HID], f32, kind="ExternalOutput")
    out_o = nc.dram_tensor("out_o", [NPC, HID], f32, kind="ExternalOutput")

    with tile.TileContext(nc) as tc:
        from contextlib import ExitStack
        with ExitStack() as ctx:
            const_p = ctx.enter_context(tc.tile_pool(name="const", bufs=1))
            meta_p = ctx.enter_context(tc.tile_pool(name="meta", bufs=1))
            big_p = ctx.enter_context(tc.tile_pool(name="big", bufs=1))
            g_p = ctx.enter_context(tc.tile_pool(name="gat", bufs=12))
            p_p = ctx.enter_context(tc.tile_pool(name="pw", bufs=8))
            nm_p = ctx.enter_context(tc.tile_pool(name="nm", bufs=4))
            sq_p = ctx.enter_context(tc.tile_pool(name="sq", bufs=2))
            st_p = ctx.enter_context(tc.tile_pool(name="st", bufs=1))
            tp_ps = ctx.enter_context(tc.tile_pool(name="tp_ps", bufs=2, space="PSUM"))
            mm_ps = ctx.enter_context(tc.tile_pool(name="mm_ps", bufs=2, space="PSUM"))
            ag_ps = ctx.enter_context(tc.tile_pool(name="ag_ps", bufs=1, space="PSUM"))
            dram_p = ctx.enter_context(tc.tile_pool(name="dram", bufs=1, space="DRAM"))

            # -------- constants / metadata loads
            iota_sb = const_p.tile([128, GMAX * 128], bf16)
            nc.sync.dma_start(out=iota_sb[:], in_=iota_t[:])
            ident_sb = const_p.tile([128, 128], bf16)
            nc.sync.dma_start(out=ident_sb[:], in_=ident_t[:])
            w_in_sb = const_p.tile([128, 2, HID], bf16)
            nc.sync.dma_start(out=w_in_sb[:], in_=w_in_t.rearrange("k p h -> p k h"))
            w_hid_sb = const_p.tile([128, HID], bf16)
            nc.sync.dma_start(out=w_hid_sb[:], in_=w_hid_t[:])
            wl1_sb = const_p.tile([128, HID], bf16)
            nc.sync.dma_start(out=wl1_sb[:], in_=wl1_t[:])
            wr1_sb = const_p.tile([128, HID], bf16)
            nc.sync.dma_start(out=wr1_sb[:], in_=wr1_t[:])
            wl2_sb = const_p.tile([128, HID], bf16)
            nc.sync.dma_start(out=wl2_sb[:], in_=wl2_t[:])
            wr2_sb = const_p.tile([128, HID], bf16)
            nc.sync.dma_start(out=wr2_sb[:], in_=wr2_t[:])
            gb_sb = const_p.tile([128, 8], f32)
            nc.sync.dma_start(out=gb_sb[:], in_=gb_t[:])

            idx_sb = meta_p.tile([128, cols], i16)
            nc.sync.dma_start(out=idx_sb[:], in_=idx_t[:])
            dloc_sb = meta_p.tile([128, tch], bf16)
            nc.sync.dma_start(out=dloc_sb[:], in_=dloc_t[:])
            invc_sb = meta_p.tile([128, NPAD], bf16)
            nc.sync.dma_start(out=invc_sb[:], in_=invc_t[:])

            # -------- DRAM internal (a/b shard split for pipelined AllGather)
            featsh_a = dram_p.tile([ASZ, HID], bf16)
            featsh_b = dram_p.tile([BSZ, HID], bf16)
            featF_a = dram_p.tile([NCORES * ASZ, HID], bf16, addr_space="Shared")
            featF_b = dram_p.tile([NCORES * BSZ, HID], bf16, addr_space="Shared")
            o1sh_a = dram_p.tile([ASZ, HID], bf16)
            o1sh_b = dram_p.tile([BSZ, HID], bf16)
            o1F_a = dram_p.tile([NCORES * ASZ, HID], bf16, addr_space="Shared")
            o1F_b = dram_p.tile([NCORES * BSZ, HID], bf16, addr_space="Shared")

            # -------- helpers
            def bn_allreduce(s_part, q_part, tag):
                st_sb = st_p.tile([128, 2], f32, tag="st_sb", bufs=2,
                                  name=f"st_sb_{tag}")
                nc.vector.tensor_reduce(st_sb[:, 0:1], s_part[:], AX, ADD)
                nc.vector.tensor_reduce(st_sb[:, 1:2], q_part[:], AX, ADD)
                st_in = dram_p.tile([128, 2], f32, tag="st_in", bufs=2,
                                    name=f"st_in_{tag}")
                st_out = dram_p.tile([128, 2], f32, tag="st_out", bufs=2,
                                     addr_space="Shared", name=f"st_out_{tag}")
                nc.sync.dma_start(out=st_in[:], in_=st_sb[:])
                nc.gpsimd.collective_compute(
                    "AllReduce", ADD, replica_groups=rg,
                    ins=[st_in.opt()], outs=[st_out.opt()])
                stg = st_p.tile([128, 2], f32, tag="stg", bufs=2,
                                name=f"stg_{tag}")
                nc.sync.dma_start(out=stg[:], in_=st_out[:])
                return stg

            def bn_coeffs(stg, gcol, tag):
                t = st_p.tile([128, 6], f32, tag="bnc", bufs=4, name=f"bnc_{tag}")
                mean, msq, var, rstd, sc, sh = (t[:, i:i + 1] for i in range(6))
                nc.vector.tensor_scalar(mean, stg[:, 0:1], inv_n, None, MUL)
                nc.vector.tensor_scalar(msq, stg[:, 1:2], inv_n, None, MUL)
                nc.vector.tensor_tensor(var, mean, mean, MUL)
                nc.vector.tensor_tensor(var, msq, var, SUB)
                std = st_p.tile([128, 1], f32, tag="bnstd", bufs=4,
                                name=f"bnstd_{tag}")
                nc.vector.tensor_scalar(var, var, float(EPS), None, ADD)
                nc.scalar.activation(std[:], var, AF.Sqrt)
                nc.vector.reciprocal(rstd, std[:])
                nc.vector.tensor_tensor(sc, rstd, gb_sb[:, gcol:gcol + 1], MUL)
                nc.vector.tensor_tensor(sh, mean, sc, MUL)
                nc.vector.tensor_tensor(sh, gb_sb[:, gcol + 1:gcol + 2], sh, SUB)
                return sc, sh

            def stats_of_psum(ps, sz, s_part, q_part, ck, tag):
                nc.vector.tensor_reduce(s_part[:, ck:ck + 1], ps[:, :sz], AX, ADD)
                sq = sq_p.tile([128, DENSE_CHUNK], f32, tag="sq", name=f"sq_{tag}")
                nc.scalar.activation(sq[:, :sz], ps[:, :sz], AF.Square,
                                     accum_out=q_part[:, ck:ck + 1])

            def emit_nm(srcT, dst32, dst16a, dst16b, lname, blo=0, bhi=NB):
                """Transpose feature-major srcT [128, NPAD] bf16 to node-major
                blocks; DMA f32 rows to dst32 and/or bf16 rows to the a/b
                shard tiles."""
                for b in range(blo, bhi):
                    b0 = b * 128
                    bs = min(128, NPC - b0)
                    tp = tp_ps.tile([128, 128], bf16, tag="tp",
                                    name=f"tp_{lname}_{b}")
                    nc.tensor.transpose(tp[:bs, :], srcT[:, b0:b0 + bs],
                                        ident_sb[:])
                    if dst32 is not None:
                        nm32 = nm_p.tile([128, 128], f32, tag="nm32",
                                         name=f"nm32_{lname}_{b}")
                        if b % 2 == 0:
                            nc.scalar.activation(nm32[:bs, :], tp[:bs, :], AF.Copy)
                        else:
                            nc.vector.tensor_copy(nm32[:bs, :], tp[:bs, :])
                        nc.sync.dma_start(out=dst32[b0:b0 + bs, :],
                                          in_=nm32[:bs, :])
                    if dst16a is not None:
                        nm16 = nm_p.tile([128, 128], bf16, tag="nm16",
                                         name=f"nm16_{lname}_{b}")
                        if b % 2 == 1:
                            nc.scalar.activation(nm16[:bs, :], tp[:bs, :], AF.Copy)
                        else:
                            nc.vector.tensor_copy(nm16[:bs, :], tp[:bs, :])
                        if b < ABLK:
                            nc.sync.dma_start(out=dst16a[b0:b0 + bs, :],
                                              in_=nm16[:bs, :])
                        else:
                            c0 = b0 - ASZ
                            nc.sync.dma_start(out=dst16b[c0:c0 + bs, :],
                                              in_=nm16[:bs, :])

            _gq = [0]

            def sage_scatter(srcF_a, srcF_b, outT, lname, on_group=None):
                """Gather + matmul-scatter accumulating meanT [feat, slot]
                feature-major in PSUM; drain multiplies by inv_cnt row tile.
                on_group(g) is invoked in program order right after all blocks
                of group g have drained, so dependent dense chunks interleave
                into the engine streams."""
                agg_tiles = {}
                ngrp = -(-NB // GRP)
                gdone = 0
                for (t, q0, nch) in calls:
                    gi = _gq[0]; _gq[0] += 1
                    gt = g_p.tile([128, GMAX, 128], bf16, tag="gt",
                                  name=f"gt_{lname}_{q0}")
                    src_ap = (srcF_a if t == 0 else srcF_b)[:, :]
                    nc.gpsimd.dma_gather(
                        gt[:, :nch, :], src_ap, idx_sb[:, 8 * q0:8 * (q0 + nch)],
                        nch * 128, nch * 128, elem_size=128,
                        queue_num=gi % NQ)
                    pt = p_p.tile([128, GMAX * 128], bf16, tag="pt",
                                  name=f"pt_{lname}_{q0}")
                    nc.vector.tensor_tensor(
                        pt[:, :nch * 128].rearrange("p (a b) -> p a b", a=nch),
                        iota_sb[:, :nch * 128].rearrange("p (a b) -> p a b", a=nch),
                        dloc_sb[:, q0:q0 + nch].unsqueeze(-1).broadcast_to(
                            [128, nch, 128]),
                        ISEQ)
                    for j in range(nch):
                        ci = q0 + j
                        b = stream[ci]
                        start = (ci == first_chunk[b])
                        stop = (ci == last_chunk[b])
                        if start:
                            agg_tiles[b] = ag_ps.tile(
                                [128, 128], f32, tag=f"agg{b % GRP}",
                                name=f"agg_{lname}_{b}")
                        nc.tensor.matmul(agg_tiles[b][:],
                                         lhsT=gt[:, j, :],
                                         rhs=pt[:, j * 128:(j + 1) * 128],
                                         start=start, stop=stop)
                        if stop:
                            b0 = b * 128
                            nc.vector.tensor_tensor(
                                outT[:, b0:b0 + 128], agg_tiles[b][:],
                                invc_sb[:, b0:b0 + 128], MUL)
                    if on_group is not None:
                        end = q0 + nch - 1
                        while (gdone < ngrp and
                               last_chunk[min(NB - 1, gdone * GRP + GRP - 1)] <= end):
                            on_group(gdone)
                            gdone += 1

            def dense_pair(lhs_a, rhs_aT, lhs_b, rhs_bT, outT, s_part, q_part,
                           lname):
                """outT = lhs_a^T @ rhs_aT + lhs_b^T @ rhs_bT with BN partials."""
                for ck, (c0, sz) in enumerate(cks):
                    ps = mm_ps.tile([128, DENSE_CHUNK], f32, tag="mm",
                                    name=f"mm_{lname}_{ck}")
                    nc.tensor.matmul(ps[:, :sz], lhsT=lhs_a[:],
                                     rhs=rhs_aT[:, c0:c0 + sz],
                                     start=True, stop=(lhs_b is None))
                    if lhs_b is not None:
                        nc.tensor.matmul(ps[:, :sz], lhsT=lhs_b[:],
                                         rhs=rhs_bT[:, c0:c0 + sz],
                                         start=False, stop=True)
                    stats_of_psum(ps, sz, s_part, q_part, ck, f"{lname}_{ck}")
                    nc.scalar.activation(outT[:, c0:c0 + sz], ps[:, :sz], AF.Copy)

            # ================= Phase A: input layer =================
            _sid, _ = nc.enter_named_scope("phA", False)
            xt0 = big_p.tile([128, NPAD], bf16, tag="A", name="xt0")
            xt1 = big_p.tile([128, NPAD], bf16, tag="B", name="xt1")
            h1T = big_p.tile([128, NPAD], bf16, tag="C", name="h1T")
            s1 = st_p.tile([128, nck], f32, tag="sp", bufs=2, name="s1")
            q1 = st_p.tile([128, nck], f32, tag="qp", bufs=2, name="q1")
            for (c0, sz) in xks:
                nc.sync.dma_start_transpose(out=xt0[:, c0:c0 + sz],
                                            in_=xs_t[0, c0:c0 + sz, :])
                nc.sync.dma_start_transpose(out=xt1[:, c0:c0 + sz],
                                            in_=xs_t[1, c0:c0 + sz, :])
            for ck, (c0, sz) in enumerate(cks):
                ps = mm_ps.tile([128, DENSE_CHUNK], f32, tag="mm",
                                name=f"mmh1_{ck}")
                nc.tensor.matmul(ps[:, :sz], lhsT=w_in_sb[:, 0, :],
                                 rhs=xt0[:, c0:c0 + sz], start=True, stop=False)
                nc.tensor.matmul(ps[:, :sz], lhsT=w_in_sb[:, 1, :],
                                 rhs=xt1[:, c0:c0 + sz], start=False, stop=True)
                stats_of_psum(ps, sz, s1, q1, ck, f"h1_{ck}")
                nc.scalar.activation(h1T[:, c0:c0 + sz], ps[:, :sz], AF.Copy)
            stg1 = bn_allreduce(s1, q1, "bn1")
            sc1, sh1 = bn_coeffs(stg1, 0, "bn1")
            nc.leave_named_scope("phA", _sid, False)
            for (c0, sz) in cks:
                nc.scalar.activation(h1T[:, c0:c0 + sz], h1T[:, c0:c0 + sz],
                                     AF.Relu, bias=sh1, scale=sc1)

            # ================= Phase B: hidden layer =================
            _sid, _ = nc.enter_named_scope("phB", False)
            featT = big_p.tile([128, NPAD], bf16, tag="A", name="featT")
            s2 = st_p.tile([128, nck], f32, tag="sp", bufs=2, name="s2")
            q2 = st_p.tile([128, nck], f32, tag="qp", bufs=2, name="q2")
            dense_pair(w_hid_sb, h1T, None, None, featT, s2, q2, "h2")
            stg2 = bn_allreduce(s2, q2, "bn2")
            sc2, sh2 = bn_coeffs(stg2, 2, "bn2")
            nc.leave_named_scope("phB", _sid, False)

            _sid, _ = nc.enter_named_scope("agF", False)
            ck_a = (ABLK * 128 - 1) // DENSE_CHUNK
            for ck, (c0, sz) in enumerate(cks):
                nc.scalar.activation(featT[:, c0:c0 + sz], featT[:, c0:c0 + sz],
                                     AF.Relu, bias=sh2, scale=sc2)
                blo = c0 // 128
                bhi = min(NB, (c0 + sz + 127) // 128)
                emit_nm(featT, feat_o, featsh_a, featsh_b, "f",
                        blo, min(bhi, ABLK))
                if ck == ck_a:
                    nc.gpsimd.collective_compute(
                        "AllGather", BYP, replica_groups=rg,
                        ins=[featsh_a.opt()], outs=[featF_a.opt()])
            for ck, (c0, sz) in enumerate(cks):
                blo = max(ABLK, c0 // 128)
                bhi = min(NB, (c0 + sz + 127) // 128)
                if blo < bhi:
                    emit_nm(featT, feat_o, featsh_a, featsh_b, "f", blo, bhi)
            nc.gpsimd.collective_compute(
                "AllGather", BYP, replica_groups=rg,
                ins=[featsh_b.opt()], outs=[featF_b.opt()])
            nc.leave_named_scope("agF", _sid, False)

            # ================= SAGE layer 1 (+ interleaved dn1) ==========
            _sid, _ = nc.enter_named_scope("sc1", False)
            meanT = big_p.tile([128, NPAD], bf16, tag="B", name="meanT")
            out1T = big_p.tile([128, NPAD], bf16, tag="C", name="out1T")
            s3 = st_p.tile([128, nck], f32, tag="sp", bufs=2, name="s3")
            q3 = st_p.tile([128, nck], f32, tag="qp", bufs=2, name="q3")
            ck_a = (ABLK * 128 - 1) // DENSE_CHUNK  # chunk completing block 24

            def dn1_chunk(ck):
                c0, sz = cks[ck]
                ps = mm_ps.tile([128, DENSE_CHUNK], f32, tag="mm",
                                name=f"mm_o1_{ck}")
                nc.tensor.matmul(ps[:, :sz], lhsT=wl1_sb[:],
                                 rhs=meanT[:, c0:c0 + sz],
                                 start=True, stop=False)
                nc.tensor.matmul(ps[:, :sz], lhsT=wr1_sb[:],
                                 rhs=featT[:, c0:c0 + sz],
                                 start=False, stop=True)
                stats_of_psum(ps, sz, s3, q3, ck, f"o1_{ck}")
                nc.scalar.activation(out1T[:, c0:c0 + sz], ps[:, :sz], AF.Copy)
                if ck == ck_a:
                    emit_nm(out1T, None, o1sh_a, o1sh_b, "o1", 0, ABLK)
                    nc.gpsimd.collective_compute(
                        "AllGather", BYP, replica_groups=rg,
                        ins=[o1sh_a.opt()], outs=[o1F_a.opt()])

            sage_scatter(featF_a, featF_b, meanT, "s1", on_group=dn1_chunk)
            emit_nm(out1T, None, o1sh_a, o1sh_b, "o1", ABLK, NB)
            nc.gpsimd.collective_compute(
                "AllGather", BYP, replica_groups=rg,
                ins=[o1sh_b.opt()], outs=[o1F_b.opt()])
            stg3 = bn_allreduce(s3, q3, "bn3")
            sc3, sh3 = bn_coeffs(stg3, 4, "bn3")
            nc.leave_named_scope("sc1", _sid, False)

            # BN3 applied locally (for the Wr2 term); sc3 folded into Wl2.
            out1bn = big_p.tile([128, NPAD], bf16, tag="A", name="out1bn")
            nc.scalar.activation(out1bn[:, :NPC], out1T[:, :NPC], AF.Identity,
                                 bias=sh3, scale=sc3)
            wl2s = const_p.tile([128, HID], bf16, name="wl2s")
            nc.vector.tensor_scalar(wl2s[:], wl2_sb[:], sc3, None, MUL)

            # ================= SAGE layer 2 (+ interleaved dn2) ==========
            _sid, _ = nc.enter_named_scope("sc2", False)
            meanT2 = big_p.tile([128, NPAD], bf16, tag="B", name="meanT2")
            out2T = big_p.tile([128, NPAD], bf16, tag="C", name="out2T")
            s4 = st_p.tile([128, nck], f32, tag="sp", bufs=2, name="s4")
            q4 = st_p.tile([128, nck], f32, tag="qp", bufs=2, name="q4")

            def dn2_chunk(ck):
                c0, sz = cks[ck]
                ps = mm_ps.tile([128, DENSE_CHUNK], f32, tag="mm",
                                name=f"mm_o2_{ck}")
                nc.tensor.matmul(ps[:, :sz], lhsT=wl2s[:],
                                 rhs=meanT2[:, c0:c0 + sz],
                                 start=True, stop=False)
                nc.tensor.matmul(ps[:, :sz], lhsT=wr2_sb[:],
                                 rhs=out1bn[:, c0:c0 + sz],
                                 start=False, stop=True)
                stats_of_psum(ps, sz, s4, q4, ck, f"o2_{ck}")
                nc.scalar.activation(out2T[:, c0:c0 + sz], ps[:, :sz], AF.Copy)

            sage_scatter(o1F_a, o1F_b, meanT2, "s2", on_group=dn2_chunk)
            stg4 = bn_allreduce(s4, q4, "bn4")
            sc4, sh4 = bn_coeffs(stg4, 6, "bn4")
            nc.scalar.activation(out2T[:, :NPC], out2T[:, :NPC], AF.Identity,
                                 bias=sh4, scale=sc4)
            emit_nm(out2T, out_o, None, None, "o2")
            nc.leave_named_scope("sc2", _sid, False)

    nc.compile()
    return nc


# ---------------------------------------------------------------- runner

_CACHE = {}


def _get_program(meta):
    key = (meta["tch"], meta["calls"])
    if key not in _CACHE:
        _CACHE[key] = _build(meta)
    return _CACHE[key]


def _make_in_maps(inputs, meta, per_core):
    iota = np.broadcast_to(np.tile(np.arange(128, dtype=np.float32), GMAX),
                           (128, GMAX * 128)).astype(BF16)
    ident = np.eye(128, dtype=np.float32).astype(BF16)
    gb = np.zeros((128, 8), np.float32)
    for i, k in enumerate(["g1", "be1", "g2", "be2", "g3", "be3", "g4", "be4"]):
        gb[:, i] = np.asarray(inputs[k], np.float32)
    w_in = np.asarray(inputs["W_in"], np.float32).astype(BF16)
    shared = {
        "w_in": np.ascontiguousarray(w_in.reshape(2, 128, HID)),
        "w_hid": np.asarray(inputs["W_hid"], np.float32).astype(BF16),
        "wl1": np.asarray(inputs["Wl1"], np.float32).astype(BF16),
        "wr1": np.asarray(inputs["Wr1"], np.float32).astype(BF16),
        "wl2": np.asarray(inputs["Wl2"], np.float32).astype(BF16),
        "wr2": np.asarray(inputs["Wr2"], np.float32).astype(BF16),
        "gb": gb, "iota": np.ascontiguousarray(iota),
        "ident": np.ascontiguousarray(ident),
    }
    x = np.asarray(inputs["x"], np.float32)
    in_maps = []
    for c in range(NCORES):
        m = dict(shared)
        xs = np.zeros((2, NPAD, 128), BF16)
        xc = x[c * NPC:(c + 1) * NPC, :].astype(BF16)
        xs[0, :NPC, :] = xc[:, :128]
        xs[1, :NPC, :] = xc[:, 128:]
        m["xs"] = xs
        m.update(per_core[c])
        in_maps.append(m)
    return in_maps


def kernel(**inputs):
    from concourse.bass_utils import run_bass_kernel_spmd

    edge_index = np.asarray(inputs["edge_index"])
    meta, per_core = _prep(edge_index)
    nc = _get_program(meta)
    in_maps = _make_in_maps(inputs, meta, per_core)
    trace = bool(int(os.environ.get("KERNEL_TRACE", "0")))
    res = run_bass_kernel_spmd(nc, in_maps, list(range(NCORES)), trace=trace)
    if res.exec_time_ns is not None:
        print(f"HW exec time: {res.exec_time_ns} ns")
        if res.per_core_scope_times:
            for scope, m in res.per_core_scope_times.items():
                print(f"  scope {scope}: {m}")
        if res.instructions_and_trace is not None:
            print(f"trace: {res.instructions_and_trace[1]}")
    feat = np.concatenate([res.results[c]["feat_o"] for c in range(NCORES)], 0)
    out = np.concatenate([res.results[c]["out_o"] for c in range(NCORES)], 0)
    return (np.asarray(feat, np.float32), np.asarray(out, np.float32))


# revision 12
# speedup vs baseline: 1.0140x; 1.0140x over previous
"""Trainium2 Bass kernel for CustomStellarEncoder (2x dense+BN+relu, 2x SAGEConv+BN).

Strategy (8 NeuronCores, SPMD):
  - Nodes partitioned contiguously across cores (6250/core).
  - Activations kept feature-major ([128 feat partitions, nodes free]) in bf16; all
    dense matmuls bf16 (PSUM f32). Linear biases dropped (BatchNorm absorbs them).
  - SAGE mean-aggregation: edges bucketed host-side by (dst block of 128, src
    table a/b), padded to 128-edge chunks, batched into big dma_gather calls
    (SWDGE). Per chunk the gathered source rows G [128e x 128f] (bf16) are
    contracted on TensorE with a one-hot P [128e x 128slot] built on VectorE:
    matmul(lhsT=G, rhs=P) accumulates meanT [feat x slot] directly in PSUM
    (feature-major, no drain transpose); drain scales by inv_cnt (bf16 row tile).
  - Halo exchange: node-major bf16 shards AllGathered in two halves (table a =
    blocks 0..24, table b = blocks 25..48) so gathers can start after the first.
  - Layer-2 aggregates PRE-BN values: BN3 is affine, so A*BN3(x) =
    sc3*(A*x) + sh3; sc3 is folded into Wl2 on device and the sh3 constant is
    absorbed exactly by BN4 (all nodes have in-edges). This unhooks layer-2's
    gather from the BN3 AllReduce.
  - Global BN statistics via AllReduce of [128,2] partial (sum, sumsq).
"""

import os
import sys
import numpy as np

sys.path.insert(0, "/opt/trn_rl_repo")

import ml_dtypes


def _install_ntff_hook_shim():
    """The agent image's `antenv` lacks `axon_hooks`; bass_utils imports it
    unconditionally when trace=True under axon. Provide it, registering the
    same ctypes NTFF hook trn_agent_boot would have installed."""
    import types
    if "antenv.axon_hooks" in sys.modules:
        return
    mod = types.ModuleType("antenv.axon_hooks")
    state = {"hook": None}
    mod.set_axon_ntff_profile_hook = lambda h: state.__setitem__("hook", h)
    mod.get_axon_ntff_profile_hook = lambda: state["hook"]
    try:
        import antenv
        sys.modules["antenv.axon_hooks"] = mod
        antenv.axon_hooks = mod
        from trn_agent_boot.trn_boot import _ntff_profile_via_ctypes
        mod.set_axon_ntff_profile_hook(
            _ntff_profile_via_ctypes("/opt/axon/libaxon_pjrt.so"))
    except Exception as e:  # tracing degrades; execution still works
        print(f"ntff hook shim unavailable: {e}", file=sys.stderr)


_install_ntff_hook_shim()

N = 50000
E = 1600000
IN_DIM = 256
HID = 128
NCORES = 8
EPS = 1e-5

NPC = N // NCORES            # 6250
NB = -(-NPC // 128)          # 49 blocks
NPAD = NB * 128              # 6272
ABLK = 25                    # table-a blocks per core
ASZ = ABLK * 128             # 3200 nodes/core in table a
BSZ = NPC - ASZ              # 3050 nodes/core in table b

GRP = 4      # dst blocks whose PSUM accumulators are live together
GMAX = 8     # max 128-edge chunks per dma_gather call (HW SWDGE ring limit)
NQ = 4       # SWDGE queues
SCRATCH = 32768
DENSE_CHUNK = 512

BF16 = ml_dtypes.bfloat16


# ---------------------------------------------------------------- host prep

def _prep(edge_index):
    src = edge_index[0].astype(np.int64)
    dst = edge_index[1].astype(np.int64)

    cnt = np.bincount(dst, minlength=N)
    assert (cnt > 0).all(), "bn3 shift absorption requires all nodes to have in-edges"
    invc = (1.0 / np.maximum(cnt, 1)).astype(np.float32)

    sc_ = src // NPC
    sl = src % NPC
    tbl = (sl >= ASZ).astype(np.int64)
    lidx = np.where(tbl == 0, sc_ * ASZ + sl, sc_ * BSZ + (sl - ASZ))

    core = dst // NPC
    block = (dst % NPC) // 128
    dloc = (dst % NPC) % 128

    bucket = (core * NB + block) * 2 + tbl
    order = np.argsort(bucket, kind="stable")
    bc = np.bincount(bucket, minlength=NCORES * NB * 2).reshape(NCORES, NB, 2)
    ch = -(-bc.max(axis=0) // 128)  # [NB, 2] chunks per (block, table)

    stream = []   # chunk idx -> block
    calls = []    # (table, q0_chunk, nch)
    bstart_chunk = {}
    q = 0
    for g0 in range(0, NB, GRP):
        gb = list(range(g0, min(g0 + GRP, NB)))
        for t in (0, 1):
            run = []
            for b in gb:
                bstart_chunk[(b, t)] = q + len(run)
                run += [b] * int(ch[b, t])
            off = 0
            while off < len(run):
                c = min(GMAX, len(run) - off)
                calls.append((t, q + off, c))
                off += c
            stream += run
            q += len(run)
    tch = q
    first_chunk = {}
    last_chunk = {}
    for ci, b in enumerate(stream):
        first_chunk.setdefault(b, ci)
        last_chunk[b] = ci

    bstarts = np.zeros(NCORES * NB * 2 + 1, np.int64)
    np.cumsum(bc.reshape(-1), out=bstarts[1:])

    slots = tch * 128
    idx_all = np.zeros((NCORES, slots), np.int16)
    dl_all = np.full((NCORES, slots), 200.0, np.float32)
    for c in range(NCORES):
        for b in range(NB):
            for t in (0, 1):
                bid = (c * NB + b) * 2 + t
                e0, e1 = int(bstarts[bid]), int(bstarts[bid + 1])
                m = e1 - e0
                if m == 0:
                    continue
                so = bstart_chunk[(b, t)] * 128
                ids = order[e0:e1]
                idx_all[c, so:so + m] = lidx[ids].astype(np.int16)
                dl_all[c, so:so + m] = dloc[ids]

    per_core = []
    for c in range(NCORES):
        idx_w = np.tile(np.ascontiguousarray(idx_all[c].reshape(-1, 16).T), (8, 1))
        ivp = np.zeros(NPAD, np.float32)
        ivp[:NPC] = invc[c * NPC:(c + 1) * NPC]
        invc_bc = np.broadcast_to(ivp[None, :], (128, NPAD)).astype(BF16)
        per_core.append({
            "idx": np.ascontiguousarray(idx_w),
            "dloc": np.ascontiguousarray(dl_all[c].reshape(tch, 128).T).astype(BF16),
            "invc": np.ascontiguousarray(invc_bc),
        })

    meta = {
        "tch": tch,
        "calls": tuple(calls),
        "stream": tuple(stream),
        "first_chunk": first_chunk,
        "last_chunk": last_chunk,
    }
    return meta, per_core


# ---------------------------------------------------------------- device build

def _build(meta):
    import concourse.bacc as bacc
    import concourse.tile as tile
    from concourse import mybir

    f32 = mybir.dt.float32
    bf16 = mybir.dt.bfloat16
    i16 = mybir.dt.int16
    ADD = mybir.AluOpType.add
    MUL = mybir.AluOpType.mult
    SUB = mybir.AluOpType.subtract
    ISEQ = mybir.AluOpType.is_equal
    BYP = mybir.AluOpType.bypass
    AX = mybir.AxisListType.X
    AF = mybir.ActivationFunctionType

    tch = meta["tch"]
    calls = meta["calls"]
    stream = meta["stream"]
    first_chunk = meta["first_chunk"]
    last_chunk = meta["last_chunk"]
    cols = tch * 8
    inv_n = 1.0 / N
    rg = [list(range(NCORES))]

    cks = [(i, min(DENSE_CHUNK, NPC - i)) for i in range(0, NPC, DENSE_CHUNK)]
    nck = len(cks)
    # x-transpose chunks cover the padded 6272 rows
    xks = [(i, min(1568, NPAD - i)) for i in range(0, NPAD, 1568)]

    nc = bacc.Bacc("TRN2", target_bir_lowering=False, debug=False,
                   num_devices=NCORES, num_swdge_queues=NQ,
                   dynamic_dma_scratch_size=SCRATCH)

    # -------- I/O
    xs_t = nc.dram_tensor("xs", [2, NPAD, 128], bf16, kind="ExternalInput")
    idx_t = nc.dram_tensor("idx", [128, cols], i16, kind="ExternalInput")
    dloc_t = nc.dram_tensor("dloc", [128, tch], bf16, kind="ExternalInput")
    invc_t = nc.dram_tensor("invc", [128, NPAD], bf16, kind="ExternalInput")
    w_in_t = nc.dram_tensor("w_in", [2, 128, HID], bf16, kind="ExternalInput")
    w_hid_t = nc.dram_tensor("w_hid", [HID, HID], bf16, kind="ExternalInput")
    wl1_t = nc.dram_tensor("wl1", [HID, HID], bf16, kind="ExternalInput")
    wr1_t = nc.dram_tensor("wr1", [HID, HID], bf16, kind="ExternalInput")
    wl2_t = nc.dram_tensor("wl2", [HID, HID], bf16, kind="ExternalInput")
    wr2_t = nc.dram_tensor("wr2", [HID, HID], bf16, kind="ExternalInput")
    gb_t = nc.dram_tensor("gb", [128, 8], f32, kind="ExternalInput")
    iota_t = nc.dram_tensor("iota", [128, GMAX * 128], bf16, kind="ExternalInput")
    ident_t = nc.dram_tensor("ident", [128, 128], bf16, kind="ExternalInput")
    feat_o = nc.dram_tensor("feat_o", [NPC, HID], f32, kind="ExternalOutput")
    out_o = nc.dram_tensor("out_o", [NPC, HID], f32, kind="ExternalOutput")

    with tile.TileContext(nc) as tc:
        from contextlib import ExitStack
        with ExitStack() as ctx:
            const_p = ctx.enter_context(tc.tile_pool(name="const", bufs=1))
            meta_p = ctx.enter_context(tc.tile_pool(name="meta", bufs=1))
            big_p = ctx.enter_context(tc.tile_pool(name="big", bufs=1))
            g_p = ctx.enter_context(tc.tile_pool(name="gat", bufs=16))
            p_p = ctx.enter_context(tc.tile_pool(name="pw", bufs=10))
            nm_p = ctx.enter_context(tc.tile_pool(name="nm", bufs=4))
            sq_p = ctx.enter_context(tc.tile_pool(name="sq", bufs=2))
            st_p = ctx.enter_context(tc.tile_pool(name="st", bufs=1))
            tp_ps = ctx.enter_context(tc.tile_pool(name="tp_ps", bufs=2, space="PSUM"))
            mm_ps = ctx.enter_context(tc.tile_pool(name="mm_ps", bufs=2, space="PSUM"))
            ag_ps = ctx.enter_context(tc.tile_pool(name="ag_ps", bufs=1, space="PSUM"))
            dram_p = ctx.enter_context(tc.tile_pool(name="dram", bufs=1, space="DRAM"))

            # -------- constants / metadata loads
            iota_sb = const_p.tile([128, GMAX * 128], bf16)
            nc.sync.dma_start(out=iota_sb[:], in_=iota_t[:])
            ident_sb = const_p.tile([128, 128], bf16)
            nc.sync.dma_start(out=ident_sb[:], in_=ident_t[:])
            w_in_sb = const_p.tile([128, 2, HID], bf16)
            nc.sync.dma_start(out=w_in_sb[:], in_=w_in_t.rearrange("k p h -> p k h"))
            w_hid_sb = const_p.tile([128, HID], bf16)
            nc.sync.dma_start(out=w_hid_sb[:], in_=w_hid_t[:])
            wl1_sb = const_p.tile([128, HID], bf16)
            nc.sync.dma_start(out=wl1_sb[:], in_=wl1_t[:])
            wr1_sb = const_p.tile([128, HID], bf16)
            nc.sync.dma_start(out=wr1_sb[:], in_=wr1_t[:])
            wl2_sb = const_p.tile([128, HID], bf16)
            nc.sync.dma_start(out=wl2_sb[:], in_=wl2_t[:])
            wr2_sb = const_p.tile([128, HID], bf16)
            nc.sync.dma_start(out=wr2_sb[:], in_=wr2_t[:])
            gb_sb = const_p.tile([128, 8], f32)
            nc.sync.dma_start(out=gb_sb[:], in_=gb_t[:])

            idx_sb = meta_p.tile([128, cols], i16)
            nc.sync.dma_start(out=idx_sb[:], in_=idx_t[:])
            dloc_sb = meta_p.tile([128, tch], bf16)
            nc.sync.dma_start(out=dloc_sb[:], in_=dloc_t[:])
            invc_sb = meta_p.tile([128, NPAD], bf16)
            nc.sync.dma_start(out=invc_sb[:], in_=invc_t[:])

            # -------- DRAM internal (a/b shard split for pipelined AllGather)
            featsh_a = dram_p.tile([ASZ, HID], bf16)
            featsh_b = dram_p.tile([BSZ, HID], bf16)
            featF_a = dram_p.tile([NCORES * ASZ, HID], bf16, addr_space="Shared")
            featF_b = dram_p.tile([NCORES * BSZ, HID], bf16, addr_space="Shared")
            o1sh_a = dram_p.tile([ASZ, HID], bf16)
            o1sh_b = dram_p.tile([BSZ, HID], bf16)
            o1F_a = dram_p.tile([NCORES * ASZ, HID], bf16, addr_space="Shared")
            o1F_b = dram_p.tile([NCORES * BSZ, HID], bf16, addr_space="Shared")

            # -------- helpers
            def bn_allreduce(s_part, q_part, tag):
                st_sb = st_p.tile([128, 2], f32, tag="st_sb", bufs=2,
                                  name=f"st_sb_{tag}")
                nc.vector.tensor_reduce(st_sb[:, 0:1], s_part[:], AX, ADD)
                nc.vector.tensor_reduce(st_sb[:, 1:2], q_part[:], AX, ADD)
                st_in = dram_p.tile([128, 2], f32, tag="st_in", bufs=2,
                                    name=f"st_in_{tag}")
                st_out = dram_p.tile([128, 2], f32, tag="st_out", bufs=2,
                                     addr_space="Shared", name=f"st_out_{tag}")
                nc.sync.dma_start(out=st_in[:], in_=st_sb[:])
                nc.gpsimd.collective_compute(
                    "AllReduce", ADD, replica_groups=rg,
                    ins=[st_in.opt()], outs=[st_out.opt()])
                stg = st_p.tile([128, 2], f32, tag="stg", bufs=2,
                                name=f"stg_{tag}")
                nc.sync.dma_start(out=stg[:], in_=st_out[:])
                return stg

            def bn_coeffs(stg, gcol, tag):
                t = st_p.tile([128, 6], f32, tag="bnc", bufs=4, name=f"bnc_{tag}")
                mean, msq, var, rstd, sc, sh = (t[:, i:i + 1] for i in range(6))
                nc.vector.tensor_scalar(mean, stg[:, 0:1], inv_n, None, MUL)
                nc.vector.tensor_scalar(msq, stg[:, 1:2], inv_n, None, MUL)
                nc.vector.tensor_tensor(var, mean, mean, MUL)
                nc.vector.tensor_tensor(var, msq, var, SUB)
                std = st_p.tile([128, 1], f32, tag="bnstd", bufs=4,
                                name=f"bnstd_{tag}")
                nc.vector.tensor_scalar(var, var, float(EPS), None, ADD)
                nc.scalar.activation(std[:], var, AF.Sqrt)
                nc.vector.reciprocal(rstd, std[:])
                nc.vector.tensor_tensor(sc, rstd, gb_sb[:, gcol:gcol + 1], MUL)
                nc.vector.tensor_tensor(sh, mean, sc, MUL)
                nc.vector.tensor_tensor(sh, gb_sb[:, gcol + 1:gcol + 2], sh, SUB)
                return sc, sh

            def stats_of_psum(ps, sz, s_part, q_part, ck, tag):
                nc.vector.tensor_reduce(s_part[:, ck:ck + 1], ps[:, :sz], AX, ADD)
                sq = sq_p.tile([128, DENSE_CHUNK], f32, tag="sq", name=f"sq_{tag}")
                nc.scalar.activation(sq[:, :sz], ps[:, :sz], AF.Square,
                                     accum_out=q_part[:, ck:ck + 1])

            def emit_nm(srcT, dst32, dst16a, dst16b, lname, blo=0, bhi=NB):
                """Transpose feature-major srcT [128, NPAD] bf16 to node-major
                blocks; DMA f32 rows to dst32 and/or bf16 rows to the a/b
                shard tiles."""
                for b in range(blo, bhi):
                    b0 = b * 128
                    bs = min(128, NPC - b0)
                    tp = tp_ps.tile([128, 128], bf16, tag="tp",
                                    name=f"tp_{lname}_{b}")
                    nc.tensor.transpose(tp[:bs, :], srcT[:, b0:b0 + bs],
                                        ident_sb[:])
                    if dst32 is not None:
                        nm32 = nm_p.tile([128, 128], f32, tag="nm32",
                                         name=f"nm32_{lname}_{b}")
                        if b % 2 == 0:
                            nc.scalar.activation(nm32[:bs, :], tp[:bs, :], AF.Copy)
                        else:
                            nc.vector.tensor_copy(nm32[:bs, :], tp[:bs, :])
                        nc.sync.dma_start(out=dst32[b0:b0 + bs, :],
                                          in_=nm32[:bs, :])
                    if dst16a is not None:
                        nm16 = nm_p.tile([128, 128], bf16, tag="nm16",
                                         name=f"nm16_{lname}_{b}")
                        if b % 2 == 1:
                            nc.scalar.activation(nm16[:bs, :], tp[:bs, :], AF.Copy)
                        else:
                            nc.vector.tensor_copy(nm16[:bs, :], tp[:bs, :])
                        if b < ABLK:
                            nc.sync.dma_start(out=dst16a[b0:b0 + bs, :],
                                              in_=nm16[:bs, :])
                        else:
                            c0 = b0 - ASZ
                            nc.sync.dma_start(out=dst16b[c0:c0 + bs, :],
                                              in_=nm16[:bs, :])

            _gq = [0]

            def sage_scatter(srcF_a, srcF_b, outT, lname, on_group=None):
                """Gather + matmul-scatter accumulating meanT [feat, slot]
                feature-major in PSUM; drain multiplies by inv_cnt row tile.
                on_group(g) is invoked in program order right after all blocks
                of group g have drained, so dependent dense chunks interleave
                into the engine streams."""
                agg_tiles = {}
                ngrp = -(-NB // GRP)
                gdone = 0
                for (t, q0, nch) in calls:
                    gi = _gq[0]; _gq[0] += 1
                    gt = g_p.tile([128, GMAX, 128], bf16, tag="gt",
                                  name=f"gt_{lname}_{q0}")
                    src_ap = (srcF_a if t == 0 else srcF_b)[:, :]
                    nc.gpsimd.dma_gather(
                        gt[:, :nch, :], src_ap, idx_sb[:, 8 * q0:8 * (q0 + nch)],
                        nch * 128, nch * 128, elem_size=128,
                        queue_num=gi % NQ)
                    pt = p_p.tile([128, GMAX * 128], bf16, tag="pt",
                                  name=f"pt_{lname}_{q0}")
                    nc.vector.tensor_tensor(
                        pt[:, :nch * 128].rearrange("p (a b) -> p a b", a=nch),
                        iota_sb[:, :nch * 128].rearrange("p (a b) -> p a b", a=nch),
                        dloc_sb[:, q0:q0 + nch].unsqueeze(-1).broadcast_to(
                            [128, nch, 128]),
                        ISEQ)
                    for j in range(nch):
                        ci = q0 + j
                        b = stream[ci]
                        start = (ci == first_chunk[b])
                        stop = (ci == last_chunk[b])
                        if start:
                            agg_tiles[b] = ag_ps.tile(
                                [128, 128], f32, tag=f"agg{b % GRP}",
                                name=f"agg_{lname}_{b}")
                        nc.tensor.matmul(agg_tiles[b][:],
                                         lhsT=gt[:, j, :],
                                         rhs=pt[:, j * 128:(j + 1) * 128],
                                         start=start, stop=stop)
                        if stop:
                            b0 = b * 128
                            nc.vector.tensor_tensor(
                                outT[:, b0:b0 + 128], agg_tiles[b][:],
                                invc_sb[:, b0:b0 + 128], MUL)
                    if on_group is not None:
                        end = q0 + nch - 1
                        while (gdone < ngrp and
                               last_chunk[min(NB - 1, gdone * GRP + GRP - 1)] <= end):
                            on_group(gdone)
                            gdone += 1

            def dense_pair(lhs_a, rhs_aT, lhs_b, rhs_bT, outT, s_part, q_part,
                           lname):
                """outT = lhs_a^T @ rhs_aT + lhs_b^T @ rhs_bT with BN partials."""
                for ck, (c0, sz) in enumerate(cks):
                    ps = mm_ps.tile([128, DENSE_CHUNK], f32, tag="mm",
                                    name=f"mm_{lname}_{ck}")
                    nc.tensor.matmul(ps[:, :sz], lhsT=lhs_a[:],
                                     rhs=rhs_aT[:, c0:c0 + sz],
                                     start=True, stop=(lhs_b is None))
                    if lhs_b is not None:
                        nc.tensor.matmul(ps[:, :sz], lhsT=lhs_b[:],
                                         rhs=rhs_bT[:, c0:c0 + sz],
                                         start=False, stop=True)
                    stats_of_psum(ps, sz, s_part, q_part, ck, f"{lname}_{ck}")
                    nc.scalar.activation(outT[:, c0:c0 + sz], ps[:, :sz], AF.Copy)

            # ================= Phase A: input layer =================
            _sid, _ = nc.enter_named_scope("phA", False)
            xt0 = big_p.tile([128, NPAD], bf16, tag="A", name="xt0")
            xt1 = big_p.tile([128, NPAD], bf16, tag="B", name="xt1")
            h1T = big_p.tile([128, NPAD], bf16, tag="C", name="h1T")
            s1 = st_p.tile([128, nck], f32, tag="sp", bufs=2, name="s1")
            q1 = st_p.tile([128, nck], f32, tag="qp", bufs=2, name="q1")
            for (c0, sz) in xks:
                nc.sync.dma_start_transpose(out=xt0[:, c0:c0 + sz],
                                            in_=xs_t[0, c0:c0 + sz, :])
                nc.sync.dma_start_transpose(out=xt1[:, c0:c0 + sz],
                                            in_=xs_t[1, c0:c0 + sz, :])
            for ck, (c0, sz) in enumerate(cks):
                ps = mm_ps.tile([128, DENSE_CHUNK], f32, tag="mm",
                                name=f"mmh1_{ck}")
                nc.tensor.matmul(ps[:, :sz], lhsT=w_in_sb[:, 0, :],
                                 rhs=xt0[:, c0:c0 + sz], start=True, stop=False)
                nc.tensor.matmul(ps[:, :sz], lhsT=w_in_sb[:, 1, :],
                                 rhs=xt1[:, c0:c0 + sz], start=False, stop=True)
                stats_of_psum(ps, sz, s1, q1, ck, f"h1_{ck}")
                nc.scalar.activation(h1T[:, c0:c0 + sz], ps[:, :sz], AF.Copy)
            stg1 = bn_allreduce(s1, q1, "bn1")
            sc1, sh1 = bn_coeffs(stg1, 0, "bn1")
            nc.leave_named_scope("phA", _sid, False)
            nc.scalar.activation(h1T[:, :NPC], h1T[:, :NPC], AF.Relu,
                                 bias=sh1, scale=sc1)

            # ================= Phase B: hidden layer =================
            _sid, _ = nc.enter_named_scope("phB", False)
            featT = big_p.tile([128, NPAD], bf16, tag="A", name="featT")
            s2 = st_p.tile([128, nck], f32, tag="sp", bufs=2, name="s2")
            q2 = st_p.tile([128, nck], f32, tag="qp", bufs=2, name="q2")
            dense_pair(w_hid_sb, h1T, None, None, featT, s2, q2, "h2")
            stg2 = bn_allreduce(s2, q2, "bn2")
            sc2, sh2 = bn_coeffs(stg2, 2, "bn2")
            nc.scalar.activation(featT[:, :NPC], featT[:, :NPC], AF.Relu,
                                 bias=sh2, scale=sc2)
            nc.leave_named_scope("phB", _sid, False)

            _sid, _ = nc.enter_named_scope("agF", False)
            emit_nm(featT, feat_o, featsh_a, featsh_b, "f")
            nc.gpsimd.collective_compute(
                "AllGather", BYP, replica_groups=rg,
                ins=[featsh_a.opt()], outs=[featF_a.opt()])
            nc.gpsimd.collective_compute(
                "AllGather", BYP, replica_groups=rg,
                ins=[featsh_b.opt()], outs=[featF_b.opt()])
            nc.leave_named_scope("agF", _sid, False)

            # ================= SAGE layer 1 (+ interleaved dn1) ==========
            _sid, _ = nc.enter_named_scope("sc1", False)
            meanT = big_p.tile([128, NPAD], bf16, tag="B", name="meanT")
            out1T = big_p.tile([128, NPAD], bf16, tag="C", name="out1T")
            s3 = st_p.tile([128, nck], f32, tag="sp", bufs=2, name="s3")
            q3 = st_p.tile([128, nck], f32, tag="qp", bufs=2, name="q3")
            ck_a = (ABLK * 128 - 1) // DENSE_CHUNK  # chunk completing block 24

            def dn1_chunk(ck):
                c0, sz = cks[ck]
                ps = mm_ps.tile([128, DENSE_CHUNK], f32, tag="mm",
                                name=f"mm_o1_{ck}")
                nc.tensor.matmul(ps[:, :sz], lhsT=wl1_sb[:],
                                 rhs=meanT[:, c0:c0 + sz],
                                 start=True, stop=False)
                nc.tensor.matmul(ps[:, :sz], lhsT=wr1_sb[:],
                                 rhs=featT[:, c0:c0 + sz],
                                 start=False, stop=True)
                stats_of_psum(ps, sz, s3, q3, ck, f"o1_{ck}")
                nc.scalar.activation(out1T[:, c0:c0 + sz], ps[:, :sz], AF.Copy)
                if ck == ck_a:
                    emit_nm(out1T, None, o1sh_a, o1sh_b, "o1", 0, ABLK)
                    nc.gpsimd.collective_compute(
                        "AllGather", BYP, replica_groups=rg,
                        ins=[o1sh_a.opt()], outs=[o1F_a.opt()])

            sage_scatter(featF_a, featF_b, meanT, "s1", on_group=dn1_chunk)
            emit_nm(out1T, None, o1sh_a, o1sh_b, "o1", ABLK, NB)
            nc.gpsimd.collective_compute(
                "AllGather", BYP, replica_groups=rg,
                ins=[o1sh_b.opt()], outs=[o1F_b.opt()])
            stg3 = bn_allreduce(s3, q3, "bn3")
            sc3, sh3 = bn_coeffs(stg3, 4, "bn3")
            nc.leave_named_scope("sc1", _sid, False)

            # BN3 applied locally (for the Wr2 term); sc3 folded into Wl2.
            out1bn = big_p.tile([128, NPAD], bf16, tag="A", name="out1bn")
            nc.scalar.activation(out1bn[:, :NPC], out1T[:, :NPC], AF.Identity,
                                 bias=sh3, scale=sc3)
            wl2s = const_p.tile([128, HID], bf16, name="wl2s")
            nc.vector.tensor_scalar(wl2s[:], wl2_sb[:], sc3, None, MUL)

            # ================= SAGE layer 2 (+ interleaved dn2) ==========
            _sid, _ = nc.enter_named_scope("sc2", False)
            meanT2 = big_p.tile([128, NPAD], bf16, tag="B", name="meanT2")
            out2T = big_p.tile([128, NPAD], bf16, tag="C", name="out2T")
            s4 = st_p.tile([128, nck], f32, tag="sp", bufs=2, name="s4")
            q4 = st_p.tile([128, nck], f32, tag="qp", bufs=2, name="q4")

            def dn2_chunk(ck):
                c0, sz = cks[ck]
                ps = mm_ps.tile([128, DENSE_CHUNK], f32, tag="mm",
                                name=f"mm_o2_{ck}")
                nc.tensor.matmul(ps[:, :sz], lhsT=wl2s[:],
                                 rhs=meanT2[:, c0:c0 + sz],
                                 start=True, stop=False)
                nc.tensor.matmul(ps[:, :sz], lhsT=wr2_sb[:],
                                 rhs=out1bn[:, c0:c0 + sz],
                                 start=False, stop=True)
                stats_of_psum(ps, sz, s4, q4, ck, f"o2_{ck}")
                nc.scalar.activation(out2T[:, c0:c0 + sz], ps[:, :sz], AF.Copy)

            sage_scatter(o1F_a, o1F_b, meanT2, "s2", on_group=dn2_chunk)
            stg4 = bn_allreduce(s4, q4, "bn4")
            sc4, sh4 = bn_coeffs(stg4, 6, "bn4")
            nc.scalar.activation(out2T[:, :NPC], out2T[:, :NPC], AF.Identity,
                                 bias=sh4, scale=sc4)
            emit_nm(out2T, out_o, None, None, "o2")
            nc.leave_named_scope("sc2", _sid, False)

    nc.compile()
    return nc


# ---------------------------------------------------------------- runner

_CACHE = {}


def _get_program(meta):
    key = (meta["tch"], meta["calls"])
    if key not in _CACHE:
        _CACHE[key] = _build(meta)
    return _CACHE[key]


def _make_in_maps(inputs, meta, per_core):
    iota = np.broadcast_to(np.tile(np.arange(128, dtype=np.float32), GMAX),
                           (128, GMAX * 128)).astype(BF16)
    ident = np.eye(128, dtype=np.float32).astype(BF16)
    gb = np.zeros((128, 8), np.float32)
    for i, k in enumerate(["g1", "be1", "g2", "be2", "g3", "be3", "g4", "be4"]):
        gb[:, i] = np.asarray(inputs[k], np.float32)
    w_in = np.asarray(inputs["W_in"], np.float32).astype(BF16)
    shared = {
        "w_in": np.ascontiguousarray(w_in.reshape(2, 128, HID)),
        "w_hid": np.asarray(inputs["W_hid"], np.float32).astype(BF16),
        "wl1": np.asarray(inputs["Wl1"], np.float32).astype(BF16),
        "wr1": np.asarray(inputs["Wr1"], np.float32).astype(BF16),
        "wl2": np.asarray(inputs["Wl2"], np.float32).astype(BF16),
        "wr2": np.asarray(inputs["Wr2"], np.float32).astype(BF16),
        "gb": gb, "iota": np.ascontiguousarray(iota),
        "ident": np.ascontiguousarray(ident),
    }
    x = np.asarray(inputs["x"], np.float32)
    in_maps = []
    for c in range(NCORES):
        m = dict(shared)
        xs = np.zeros((2, NPAD, 128), BF16)
        xc = x[c * NPC:(c + 1) * NPC, :].astype(BF16)
        xs[0, :NPC, :] = xc[:, :128]
        xs[1, :NPC, :] = xc[:, 128:]
        m["xs"] = xs
        m.update(per_core[c])
        in_maps.append(m)
    return in_maps


def kernel(**inputs):
    from concourse.bass_utils import run_bass_kernel_spmd

    edge_index = np.asarray(inputs["edge_index"])
    meta, per_core = _prep(edge_index)
    nc = _get_program(meta)
    in_maps = _make_in_maps(inputs, meta, per_core)
    trace = bool(int(os.environ.get("KERNEL_TRACE", "0")))
    res = run_bass_kernel_spmd(nc, in_maps, list(range(NCORES)), trace=trace)
    if res.exec_time_ns is not None:
        print(f"HW exec time: {res.exec_time_ns} ns")
        if res.per_core_scope_times:
            for scope, m in res.per_core_scope_times.items():
                print(f"  scope {scope}: {m}")
        if res.instructions_and_trace is not None:
            print(f"trace: {res.instructions_and_trace[1]}")
    feat = np.concatenate([res.results[c]["feat_o"] for c in range(NCORES)], 0)
    out = np.concatenate([res.results[c]["out_o"] for c in range(NCORES)], 0)
    return (np.asarray(feat, np.float32), np.asarray(out, np.float32))
